# revision 1
# baseline (speedup 1.0000x reference)
"""Trainium2 Bass kernel for nn_CustomPuzzleLoss (histogram_binning).

Computes, over preds f32[26214400] and targets i32[26214400] (1,048,576
puzzle grids of 5x5):
  loss1 = mean(|preds - targets|)
  loss2 = 0.1 * (# elements equal to an earlier element in their grid row
                 + same for grid columns) / n_grids
  oob   = any(preds < 0.5 | preds > 5.5) -> +1000.0

Sharding: pure data-parallel over the grid dimension; each of the 8 cores
processes a contiguous 131,072-grid slice laid out as 128 SBUF partitions
x 25,600 elements, streamed in 10 chunks ([800, 3200*7, 1600, 800]
elements per partition; the small first chunk starts compute early and
the tapered tail shortens the post-stream drain).

The duplicate counting needs 100 pairwise equality checks per 25-element
grid (all ordered pairs within each row and each column, distances
d=1..4) — 4x the input volume, too much for the vector engine alone
(~123 G elem/s f32 two-source). The Pool engine cannot run comparison
ops at all (walrus rejects is_equal/is_lt on Pool), so the work splits:

  PDIFF classes (r1,c1,r2,r4,c4 = 65 units): Pool computes the strided
      pair DIFFERENCE a-b (subtract is Pool-legal) into a dense f32
      tile; DVE counts zeros with a one-source `tensor_scalar is_equal
      0` + add-reduce accumulator, which with dense SBUF operands
      qualifies for the DVE 2x perf mode (2 elem/cycle).
      (a-b==0 iff a==b exactly: gaussian-range f32 preds differ by
      >= ~3e-8 when distinct — no subnormal underflow.)
  DIRECT classes (c2,r3,c3 = 35 units): DVE two-source
      scalar_tensor_tensor is_equal at 1x on strided grid views.
  ACT: sum |p-t| (Abs, fused accum) plus both out-of-bounds checks
      (Relu(p-5.5) and Relu(0.5-p) sums).
  Pool also computes d = p - t (int32 upcast) feeding ACT's Abs.

NOTE: two modeled-faster variants were tried and rejected on hardware
evidence: bf16 diff tiles + DVE 4x zero-counts (87.7 us modeled) crash
the exec unit (NRT_EXEC_UNIT_UNRECOVERABLE) despite passing CoreSim AND
walrus compile; keep diffs f32/2x. This f32/2x design is HW-validated.

Modeled (CoreSim) per-core device time ~89.6 us vs an ~82.5 us pure-DMA
floor (26.2 MB at the modeled ~318 GB/s; the cost model serializes all
DMA transfers on one global resource, so the floor is hard). Engine
busy: SP/DMA 79.5, Pool 76.8, DVE 76.4, ACT 75.3 us. DVE's DIRECT ops
are emitted before its PDIFF checks so they overlap Pool's diff
production (~18 us of in-order stalls otherwise); ACT relu outputs go to
a discard tile so the next chunk's Pool subtract never waits on ACT
(WAR through the shared diff tile cost ~13 us).

Per-core output: one f32[128, 110] tensor of per-partition partial sums,
11 slots per chunk: [sum|d|, oob_hi_sum, oob_lo_sum, eq-counts x8].
The host combines in f64.

The device counts *pairs* of equal values within a row/col; the
reference counts elements equal to an earlier element (OR over earlier
positions). These agree unless some value appears >= 3 times in a single
row/col, which for f32 gaussian inputs has probability ~1e-14 (test.py
verifies it for the actual input: 13 duplicate pairs, no triples).
"""

import numpy as np

GRID = 5
ELEMS = GRID * GRID  # 25
N_TOTAL = 26214400
N_CORES = 8
N_PER_CORE = N_TOTAL // N_CORES  # 3,276,800
P = 128
CHUNK_PLAN = [800] + [3200] * 7 + [1600, 800]
SLOTS = 11  # 0 sum|d|, 1 oob_hi, 2 oob_lo, 3..7 PDIFF, 8..10 DIRECT
N_CHUNKS = len(CHUNK_PLAN)

# ("r", d): grid columns c vs c-d within each row; ("c", d): rows r vs r-d
# within each column. Units per grid: d=1:20, d=2:15, d=3:10, d=4:5.
PDIFF = [("r", 1), ("c", 1), ("r", 2), ("r", 4), ("c", 4)]  # 65 units
DIRECT = [("c", 2), ("r", 3), ("c", 3)]                     # 35 units

_CACHE = {}


def build_nc(n_per_core=N_PER_CORE):
    import concourse.bacc as bacc
    import concourse.mybir as mybir
    from concourse.tile import TileContext

    AF = mybir.ActivationFunctionType
    OP = mybir.AluOpType
    f32 = mybir.dt.float32

    assert n_per_core % P == 0
    f_total = n_per_core // P
    plan = CHUNK_PLAN
    assert sum(plan) == f_total and all(fc % ELEMS == 0 for fc in plan)
    fmax = max(plan)
    gmax = fmax // ELEMS

    nc = bacc.Bacc(
        "TRN2", target_bir_lowering=False, debug=False, enable_asserts=False
    )
    preds = nc.dram_tensor("preds", [n_per_core], f32, kind="ExternalInput").ap()
    targets = nc.dram_tensor(
        "targets", [n_per_core], mybir.dt.int32, kind="ExternalInput"
    ).ap()
    out = nc.dram_tensor(
        "out", [P, SLOTS * N_CHUNKS], f32, kind="ExternalOutput"
    ).ap()

    pv = preds.rearrange("(p f) -> p f", p=P)
    tv = targets.rearrange("(p f) -> p f", p=P)

    def grid_views(v, kind, d):
        if kind == "r":
            return v[:, :, :, d:], v[:, :, :, : GRID - d], GRID, GRID - d
        return v[:, :, d:, :], v[:, :, : GRID - d, :], GRID - d, GRID

    def units(kind, d):
        return GRID * (GRID - d)

    with TileContext(nc) as tc:
        with tc.tile_pool(name="work", bufs=2) as wp, \
             tc.tile_pool(name="persist", bufs=1) as pp, \
             tc.tile_pool(name="diff", bufs=2) as diff_p, \
             tc.tile_pool(name="scrv", bufs=1) as scrv_p, \
             tc.tile_pool(name="dumpa", bufs=1) as dumpa_p:
            slots = pp.tile([P, SLOTS * N_CHUNKS], f32)
            bias_hi = pp.tile([P, 1], f32)
            bias_lo = pp.tile([P, 1], f32)
            nc.vector.memset(bias_hi[:, :], -5.5)
            nc.vector.memset(bias_lo[:, :], 0.5)
            dump_a = dumpa_p.tile([P, fmax], f32, tag="dumpa", name="dumpa")
            scr_v = scrv_p.tile([P, gmax * 20], f32, tag="scrv", name="scrv")

            off = 0
            for k, fc in enumerate(plan):
                g = fc // ELEMS
                sl = slice(off, off + fc)
                off += fc
                base = SLOTS * k
                pt = wp.tile([P, fmax], f32, tag="pt", name="pt")
                tt = wp.tile([P, fmax], mybir.dt.int32, tag="tt", name="tt")
                dt_ = wp.tile([P, fmax], f32, tag="dt", name="dt")
                nc.sync.dma_start(out=pt[:, :fc], in_=pv[:, sl])
                nc.sync.dma_start(out=tt[:, :fc], in_=tv[:, sl])
                # d = p - t (int32 in1 upcast to fp32 by the ALU)
                nc.gpsimd.tensor_tensor(
                    out=dt_[:, :fc], in0=pt[:, :fc], in1=tt[:, :fc],
                    op=OP.subtract)
                # oob hi: sum relu(p - 5.5) > 0 iff any p > 5.5
                nc.scalar.activation(
                    out=dump_a[:, :fc], in_=pt[:, :fc], func=AF.Relu,
                    bias=bias_hi[:, :], scale=1.0,
                    accum_out=slots[:, base + 1 : base + 2])
                # oob lo: sum relu(0.5 - p) > 0 iff any p < 0.5
                nc.scalar.activation(
                    out=dump_a[:, :fc], in_=pt[:, :fc], func=AF.Relu,
                    bias=bias_lo[:, :], scale=-1.0,
                    accum_out=slots[:, base + 2 : base + 3])
                # sum |p - t| last: it depends on Pool's subtract, so keeping
                # it after the pt-only relus avoids in-order ACT stalls
                nc.scalar.activation(
                    out=dt_[:, :fc], in_=dt_[:, :fc], func=AF.Abs,
                    accum_out=slots[:, base : base + 1])
                v = pt[:, :fc].rearrange("p (g r c) -> p g r c", r=GRID, c=GRID)
                # Pool: strided pair-diffs into dense tiles
                dvts = []
                for i, (kind, d) in enumerate(PDIFF):
                    a, b, r_cnt, c_cnt = grid_views(v, kind, d)
                    nel = g * r_cnt * c_cnt
                    dvt = diff_p.tile([P, gmax * units(kind, d)], f32,
                                      tag=f"df{i}", name=f"df{i}")
                    dv = dvt[:, :nel].rearrange(
                        "p (g r c) -> p g r c", r=r_cnt, c=c_cnt)
                    nc.gpsimd.tensor_tensor(out=dv, in0=a, in1=b, op=OP.subtract)
                    dvts.append((dvt, nel))
                # DVE: DIRECT classes first (depend only on pt, overlap Pool)
                for j, (kind, d) in enumerate(DIRECT):
                    a, b, r_cnt, c_cnt = grid_views(v, kind, d)
                    nel = g * r_cnt * c_cnt
                    ev = scr_v[:, :nel].rearrange(
                        "p (g r c) -> p g r c", r=r_cnt, c=c_cnt)
                    nc.vector.scalar_tensor_tensor(
                        out=ev, in0=a, scalar=0.0, in1=b,
                        op0=OP.bypass, op1=OP.is_equal,
                        accum_out=slots[:, base + 8 + j : base + 9 + j])
                # DVE: 2x zero-counts over Pool's diffs
                for i, (dvt, nel) in enumerate(dvts):
                    nc.vector.tensor_scalar(
                        out=scr_v[:, :nel], in0=dvt[:, :nel], scalar1=0.0,
                        scalar2=None, op0=OP.is_equal, op1=OP.add,
                        accum_out=slots[:, base + 3 + i : base + 4 + i])

            nc.sync.dma_start(out=out, in_=slots[:, :])

    nc.compile()
    return nc


def _get_nc():
    if "nc" not in _CACHE:
        _CACHE["nc"] = build_nc()
    return _CACHE["nc"]


def make_in_maps(preds, targets):
    preds = np.ascontiguousarray(np.asarray(preds, dtype=np.float32).reshape(-1))
    targets = np.ascontiguousarray(np.asarray(targets, dtype=np.int32).reshape(-1))
    assert preds.shape == (N_TOTAL,) and targets.shape == (N_TOTAL,)
    return [
        {
            "preds": preds[c * N_PER_CORE : (c + 1) * N_PER_CORE],
            "targets": targets[c * N_PER_CORE : (c + 1) * N_PER_CORE],
        }
        for c in range(N_CORES)
    ]


def combine(results):
    """results: list of per-core dicts with the merged 'out' tensor."""
    s_abs = 0.0
    hi = 0.0
    lo = 0.0
    dup = 0.0
    for r in results:
        o = r["out"].astype(np.float64).reshape(P, N_CHUNKS, SLOTS)
        s_abs += o[:, :, 0].sum()
        hi += o[:, :, 1].sum()
        lo += o[:, :, 2].sum()
        dup += o[:, :, 3:].sum()
    loss1 = s_abs / N_TOTAL
    loss2 = dup / (N_TOTAL // ELEMS) * 0.1
    oob = (hi > 0.0) or (lo > 0.0)
    return np.asarray(loss1 + loss2 + (1000.0 if oob else 0.0), dtype=np.float32)


def kernel(preds, targets):
    from concourse import bass_utils

    nc = _get_nc()
    in_maps = make_in_maps(preds, targets)
    res = bass_utils.run_bass_kernel_spmd(
        nc, in_maps, core_ids=list(range(N_CORES))
    )
    return combine(res.results)



# revision 5
# speedup vs baseline: 16.4805x; 16.4805x over previous
"""Trainium2 Bass kernel for nn_CustomPuzzleLoss (histogram_binning).

Computes, over preds f32[26214400] and targets i32[26214400] (1,048,576
puzzle grids of 5x5):
  loss1 = mean(|preds - targets|)
  loss2 = 0.1 * (# elements equal to an earlier element in their grid row
                 + same for grid columns) / n_grids
  oob   = any(preds < 0.5 | preds > 5.5) -> +1000.0

Sharding: pure data-parallel over the grid dimension; each of the 8 cores
processes a contiguous 131,072-grid slice laid out as 128 SBUF partitions
x 25,600 elements, streamed in 4 chunks of 6,400.

Engine assignment (v2), driven by hardware slope measurements (see
test.py's methodology): per-core HBM streaming costs ~61.5 us/pass
(426 GB/s measured); DVE runs 2-source f32 ops at 1 elem/cycle/lane
@0.96 GHz; ACT at 1 elem/cycle/lane @1.2 GHz; the GPSIMD ("Pool")
engine's tensor_tensor is ~2.6 cyc/elem AND its SBUF port is shared
with DVE, so any GPSIMD work adds ~linearly to DVE time (measured:
sub-on-GPSIMD variant is +27 us vs sub-on-DVE). GPSIMD is therefore
left idle:

  DVE: all 100 pairwise equality checks per grid (8 strided
       scalar_tensor_tensor is_equal classes, accum_out counts) plus
       the d = p - t subtract (int32 in1 upcast by the ALU):
       128K cyc/partition/pass ~ 133 us -> the bottleneck.
  ACT: sum |d| (Abs, fused accum) and both out-of-bounds checks
       (Relu(p-5.5), Relu(0.5-p) sums): 76.8K cyc ~ 64 us, hidden.
  DMA: 26.2 MB/pass ~ 61.5 us, hidden under DVE.

Within each chunk the 8 compare classes are emitted before the
subtract: compares depend only on the preds DMA, so DVE never stalls
on the (independently scheduled) targets DMA.

Per-core output: one f32[128, 44] tensor of per-partition partial sums,
11 slots per chunk: [sum|d|, oob_hi_sum, oob_lo_sum, eq-counts x8].
The host combines in f64.

The device counts *pairs* of equal values within a row/col; the
reference counts elements equal to an earlier element (OR over earlier
positions). These agree unless some value appears >= 3 times in a single
row/col, which for f32 gaussian inputs has probability ~1e-14 (test.py
verifies it for the actual input: 13 duplicate pairs, no triples).
Equality is detected exactly: is_equal compares the original f32 values
(no bf16 rounding, no diff underflow — distinct gaussian-range f32
values are never closer than ~1e-20, far above the 1.4e-45 subnormal
floor).
"""

import numpy as np

GRID = 5
ELEMS = GRID * GRID  # 25
N_TOTAL = 26214400
N_CORES = 8
N_PER_CORE = N_TOTAL // N_CORES  # 3,276,800
P = 128
FC = 6400                       # elements per partition per chunk
N_CHUNKS = (N_PER_CORE // P) // FC  # 4
SLOTS = 11  # 0 sum|d|, 1 oob_hi, 2 oob_lo, 3..10 eq-class counts

# ("r", d): grid columns c vs c-d within each row; ("c", d): rows r vs r-d
# within each column. Units per grid: row d: 5*(5-d); col d: 5*(5-d).
ALL_CLASSES = [("r", 1), ("r", 2), ("r", 3), ("r", 4),
               ("c", 1), ("c", 2), ("c", 3), ("c", 4)]  # 100 pairs/grid

_CACHE = {}


def build_nc(n_per_core=N_PER_CORE, iters=1):
    """iters > 1 repeats the full streaming pass inside the NEFF, rewriting
    the same output slots each pass (output is identical to iters=1; the
    extra passes exist so a timing harness can measure marginal per-pass
    device time with dispatch overhead cancelled). kernel() uses iters=1."""
    import concourse.bacc as bacc
    import concourse.mybir as mybir
    from concourse.tile import TileContext

    AF = mybir.ActivationFunctionType
    OP = mybir.AluOpType
    f32 = mybir.dt.float32

    assert n_per_core % (P * FC) == 0
    f_total = n_per_core // P
    n_chunks = f_total // FC
    g = FC // ELEMS

    nc = bacc.Bacc(
        "TRN2", target_bir_lowering=False, debug=False, enable_asserts=False
    )
    preds = nc.dram_tensor("preds", [n_per_core], f32, kind="ExternalInput").ap()
    targets = nc.dram_tensor(
        "targets", [n_per_core], mybir.dt.int32, kind="ExternalInput"
    ).ap()
    out = nc.dram_tensor(
        "out", [P, SLOTS * n_chunks], f32, kind="ExternalOutput"
    ).ap()

    pv = preds.rearrange("(p f) -> p f", p=P)
    tv = targets.rearrange("(p f) -> p f", p=P)

    def grid_views(v, kind, d):
        if kind == "r":
            return v[:, :, :, d:], v[:, :, :, : GRID - d], GRID, GRID - d
        return v[:, :, d:, :], v[:, :, : GRID - d, :], GRID - d, GRID

    with TileContext(nc) as tc:
        with tc.tile_pool(name="work", bufs=2) as wp, \
             tc.tile_pool(name="persist", bufs=1) as pp, \
             tc.tile_pool(name="scr", bufs=1) as scr_p:
            slots = pp.tile([P, SLOTS * n_chunks], f32, name="slots")
            bias_hi = pp.tile([P, 1], f32)
            bias_lo = pp.tile([P, 1], f32)
            nc.vector.memset(bias_hi[:, :], -5.5)
            nc.vector.memset(bias_lo[:, :], 0.5)
            dump_a = scr_p.tile([P, FC], f32, tag="dumpa", name="dumpa")
            scr_v = scr_p.tile([P, g * 20], f32, tag="scrv", name="scrv")

            for k in range(n_chunks * iters):
                km = k % n_chunks
                sl = slice(km * FC, (km + 1) * FC)
                base = SLOTS * km
                pt = wp.tile([P, FC], f32, tag="pt", name="pt")
                tt = wp.tile([P, FC], mybir.dt.int32, tag="tt", name="tt")
                dt_ = wp.tile([P, FC], f32, tag="dt", name="dt")
                nc.sync.dma_start(out=pt[:, :], in_=pv[:, sl])
                nc.sync.dma_start(out=tt[:, :], in_=tv[:, sl])
                # ACT: oob checks depend only on pt
                nc.scalar.activation(
                    out=dump_a[:, :], in_=pt[:, :], func=AF.Relu,
                    bias=bias_hi[:, :], scale=1.0,
                    accum_out=slots[:, base + 1 : base + 2])
                nc.scalar.activation(
                    out=dump_a[:, :], in_=pt[:, :], func=AF.Relu,
                    bias=bias_lo[:, :], scale=-1.0,
                    accum_out=slots[:, base + 2 : base + 3])
                # DVE: the 8 compare classes first (pt-only), subtract last
                v = pt[:, :].rearrange("p (g r c) -> p g r c", r=GRID, c=GRID)
                for j, (kind, d) in enumerate(ALL_CLASSES):
                    a, b, r_cnt, c_cnt = grid_views(v, kind, d)
                    nel = g * r_cnt * c_cnt
                    ev = scr_v[:, :nel].rearrange(
                        "p (g r c) -> p g r c", r=r_cnt, c=c_cnt)
                    nc.vector.scalar_tensor_tensor(
                        out=ev, in0=a, scalar=0.0, in1=b,
                        op0=OP.bypass, op1=OP.is_equal,
                        accum_out=slots[:, base + 3 + j : base + 4 + j])
                nc.vector.tensor_tensor(
                    out=dt_[:, :], in0=pt[:, :], in1=tt[:, :], op=OP.subtract)
                # ACT: sum |d| (waits on the DVE subtract)
                nc.scalar.activation(
                    out=dt_[:, :], in_=dt_[:, :], func=AF.Abs,
                    accum_out=slots[:, base : base + 1])

            nc.sync.dma_start(out=out, in_=slots[:, :])

    nc.compile()
    return nc


def _get_nc():
    if "nc" not in _CACHE:
        _CACHE["nc"] = build_nc()
    return _CACHE["nc"]


def make_in_maps(preds, targets):
    preds = np.ascontiguousarray(np.asarray(preds, dtype=np.float32).reshape(-1))
    targets = np.ascontiguousarray(np.asarray(targets, dtype=np.int32).reshape(-1))
    assert preds.shape == (N_TOTAL,) and targets.shape == (N_TOTAL,)
    return [
        {
            "preds": preds[c * N_PER_CORE : (c + 1) * N_PER_CORE],
            "targets": targets[c * N_PER_CORE : (c + 1) * N_PER_CORE],
        }
        for c in range(N_CORES)
    ]


def combine(results):
    """results: list of per-core dicts with the merged 'out' tensor."""
    s_abs = 0.0
    hi = 0.0
    lo = 0.0
    dup = 0.0
    for r in results:
        o = r["out"].astype(np.float64).reshape(P, N_CHUNKS, SLOTS)
        s_abs += o[:, :, 0].sum()
        hi += o[:, :, 1].sum()
        lo += o[:, :, 2].sum()
        dup += o[:, :, 3:].sum()
    loss1 = s_abs / N_TOTAL
    loss2 = dup / (N_TOTAL // ELEMS) * 0.1
    oob = (hi > 0.0) or (lo > 0.0)
    return np.asarray(loss1 + loss2 + (1000.0 if oob else 0.0), dtype=np.float32)


def kernel(preds, targets):
    from concourse import bass_utils

    nc = _get_nc()
    in_maps = make_in_maps(preds, targets)
    res = bass_utils.run_bass_kernel_spmd(
        nc, in_maps, core_ids=list(range(N_CORES))
    )
    return combine(res.results)
